# revision 9
# baseline (speedup 1.0000x reference)
"""Expert-parallel BaseLayer MoE kernel for 8 TRN2 NeuronCores.

Strategy: routing (argmax over token-centroid affinities), layernorm
statistics and the sigmoid gate are computed on the host as part of the
sharding step — each core owns one expert and receives exactly the tokens
routed to it (padded to a common capacity C), pre-normalized (xhat) and
pre-transposed to [d, C].  The device does only the heavy compute:

  matmul-1 runs in fp8-e4m3 DoubleRow mode (two 128-deep k-tiles per
  instruction): pz = (64*w1')^T @ (16*xhat), epilogue
  z = max(pz + 1024*b1', 0) stored bf16 (relu commutes with the positive
  scale, which is folded into the host-side alpha/b2 constants);
  matmul-2 runs in bf16: py = w2^T z, epilogue
  out = (py + 1024*b2) * (alpha/1024) + x.

All quantization scales are powers of two so the host-side numpy
simulation is bit-identical to the device math.  Weight streams are split
across the sync and gpsimd DMA queues (w1) with the first two f-tiles as
singles for a fast start; w2 rides the sync queue as quarter-tiles once
the w1 stream drains.  No collectives; the host scatters per-expert
outputs back.
"""

import functools
import sys

import numpy as np

for _p in ("/opt/trn_rl_repo", "/opt/pypackages"):
    if _p not in sys.path:
        sys.path.append(_p)

import ml_dtypes  # noqa: E402

import concourse.bass as bass  # noqa: E402
import concourse.mybir as mybir  # noqa: E402
import concourse.tile as tile  # noqa: E402
from concourse import bacc  # noqa: E402
from concourse import bass_utils  # noqa: E402


def _ensure_axon_hooks():
    """bass_utils' trace path imports antenv.axon_hooks, which some agent
    images lack; synthesize it (with the real ctypes NTFF hook when
    available) so tracing degrades gracefully instead of crashing."""
    try:
        import antenv.axon_hooks  # noqa: F401
        return
    except ImportError:
        pass
    import types

    import antenv

    hooks = types.ModuleType("antenv.axon_hooks")
    hooks._hook = None
    hooks.set_axon_ntff_profile_hook = lambda h: setattr(hooks, "_hook", h)
    hooks.get_axon_ntff_profile_hook = lambda: hooks._hook
    sys.modules["antenv.axon_hooks"] = hooks
    antenv.axon_hooks = hooks
    try:
        from trn_agent_boot.trn_boot import _ntff_profile_via_ctypes

        hooks._hook = _ntff_profile_via_ctypes("/opt/axon/libaxon_pjrt.so")
    except Exception:
        pass


_ensure_axon_hooks()

E = 8
D = 1024
F = 4096
EPS = 1e-5
KD = D // 128   # 8 k-tiles over d
KF = F // 128   # 32 k-tiles over f
NP = KF // 2 - 1  # w1 pair-groups (f-tiles 2..31)
MAX_TC = 512    # PSUM free-dim limit for f32
SX = 16.0       # xhat fp8 scale (power of 2: lossless)
SW = 64.0       # w1 fp8 scale  (power of 2: lossless)
SK = SX * SW    # combined m1 output scale

F32 = mybir.dt.float32
BF16 = mybir.dt.bfloat16
F8 = mybir.dt.float8e4
ALU = mybir.AluOpType
AF = mybir.ActivationFunctionType
DROW = mybir.MatmulPerfMode.DoubleRow


def _chunk_slices(chunks):
    out, c0 = [], 0
    for cc in chunks:
        out.append(bass.ds(c0, cc))
        c0 += cc
    return out


def _token_chunks(c_total):
    n = (c_total + MAX_TC - 1) // MAX_TC
    base = c_total // n
    rem = c_total - base * n
    return [base + (1 if i < rem else 0) for i in range(n)]


@functools.lru_cache(maxsize=4)
def _build(c_total):
    nc = bacc.Bacc("TRN2", target_bir_lowering=False, debug=False, num_devices=E)

    # fp8 xhat (pre-scaled by SX), [128, KD, C]
    xq_d = nc.declare_dram_parameter("xq", [128, KD, c_total], F8, isOutput=False)
    # fp8 folded w1 (pre-scaled by SW): f-tiles 0,1 as singles, 2..31 as pairs
    w1a_d = nc.declare_dram_parameter("w1a", [2, 128, KD, 128], F8, isOutput=False)
    w1b_d = nc.declare_dram_parameter("w1b", [NP, 128, 2 * KD, 128], F8,
                                      isOutput=False)
    # bf16 w2, per d-tile: [KD, 128, KF, 128]
    w2_d = nc.declare_dram_parameter("w2t", [KD, 128, KF, 128], BF16, isOutput=False)
    # packed consts: col 0..KF-1 = b1K = SK*(b1 + beta@w1); col KF..KF+KD-1 = SK*b2
    cst_d = nc.declare_dram_parameter("cst", [128, KF + KD], F32, isOutput=False)
    # per-token alpha/SK row
    alr_d = nc.declare_dram_parameter("alr", [1, c_total], F32, isOutput=False)
    # f32 raw x for the residual, [128, KD, C]
    xt_d = nc.declare_dram_parameter("xt", [128, KD, c_total], F32, isOutput=False)
    out_d = nc.declare_dram_parameter("out", [KD, 128, c_total], F32, isOutput=True)

    KQ = KF // 4  # w2 quarter width (8 k2-tiles)

    with tile.TileContext(nc) as tc:
        with (
            tc.tile_pool(name="const", bufs=1) as constp,
            tc.tile_pool(name="xqp", bufs=1) as xqp,
            tc.tile_pool(name="zp", bufs=1) as zp,
            tc.tile_pool(name="w1sp", bufs=2) as w1sp,
            tc.tile_pool(name="w1p", bufs=15) as w1p,
            tc.tile_pool(name="w2p", bufs=16) as w2p,
            tc.tile_pool(name="xtp", bufs=2) as xtp,
            tc.tile_pool(name="bcast", bufs=1) as bcastp,
            tc.tile_pool(name="tmp", bufs=2) as tmpp,
            tc.tile_pool(name="outp", bufs=3) as outp,
            tc.tile_pool(name="ps_z", bufs=4, space=bass.MemorySpace.PSUM) as psz,
            tc.tile_pool(name="ps_y", bufs=2, space=bass.MemorySpace.PSUM) as psy,
            tc.tile_pool(name="ps_b", bufs=1, space=bass.MemorySpace.PSUM) as psb,
        ):
            chunks = _token_chunks(c_total)
            slices = _chunk_slices(chunks)
            nchunks = len(chunks)

            cst = constp.tile([128, KF + KD], F32, tag="cst")
            ones_lhs = constp.tile([1, 128], F32, tag="ones")
            alr = constp.tile([1, c_total], F32, tag="alr")

            for ci, cc in enumerate(chunks):
                csl = slices[ci]
                first = ci == 0

                # ---- critical-path DMAs first: xq halves on scalar+gpsimd,
                # w1 singles on sync+gpsimd, w1 pairs alternate sync/gpsimd ----
                xq = xqp.tile([128, KD, cc], F8, tag="xq")
                w1t0 = w1sp.tile([128, KD, 128], F8, tag="w1s")
                nc.sync.dma_start(out=w1t0[:], in_=w1a_d[0])
                if nchunks == 1:
                    nc.scalar.dma_start(out=xq[:, 0:KD // 2, :],
                                        in_=xq_d[:, 0:KD // 2, :])
                    nc.gpsimd.dma_start(out=xq[:, KD // 2:KD, :],
                                        in_=xq_d[:, KD // 2:KD, :])
                else:
                    nc.scalar.dma_start(out=xq[:, 0:KD // 2, :],
                                        in_=xq_d[:, 0:KD // 2, csl])
                    nc.gpsimd.dma_start(out=xq[:, KD // 2:KD, :],
                                        in_=xq_d[:, KD // 2:KD, csl])
                w1t1 = w1sp.tile([128, KD, 128], F8, tag="w1s")
                nc.gpsimd.dma_start(out=w1t1[:], in_=w1a_d[1])
                if first:
                    nc.gpsimd.dma_start(out=cst[:], in_=cst_d[:])
                    nc.gpsimd.dma_start(out=alr[:], in_=alr_d[:])
                    nc.vector.memset(ones_lhs[:], 1.0)

                z_sb = zp.tile([128, KF, cc], BF16, tag="z")
                al_b = bcastp.tile([128, cc], F32, tag="al")

                # prefetch w2 for the first two d-tiles on the scalar queue —
                # bacc hoists m2 matmuls into the m1 stream, so their weights
                # must be resident early or the tensor queue head-blocks
                w2pre = {}
                for i in range(2):
                    for h in range(4):
                        w2sb = w2p.tile([128, KQ, 128], BF16, tag="w2")
                        nc.scalar.dma_start(
                            out=w2sb[:],
                            in_=w2_d[i][:, h * KQ:(h + 1) * KQ, :],
                        )
                        w2pre[(i, h)] = w2sb

                # ---- matmul 1: fp8 DoubleRow, z = max(pz + b1K, 0) ----
                w1sb = None
                for j in range(KF):
                    if j < 2:
                        wt, jo = (w1t0, 0) if j == 0 else (w1t1, 0)
                    else:
                        p = (j - 2) // 2
                        if (j - 2) % 2 == 0:
                            w1sb = w1p.tile([128, 2 * KD, 128], F8, tag="w1")
                            eng = nc.sync if p % 2 == 0 else nc.gpsimd
                            eng.dma_start(out=w1sb[:], in_=w1b_d[p])
                        wt, jo = w1sb, ((j - 2) % 2) * KD
                    pz = psz.tile([128, cc], F32, tag="z")
                    for q in range(KD // 2):
                        nc.tensor.matmul(
                            pz[:],
                            wt[:, jo + 2 * q:jo + 2 * q + 2, :],
                            xq[:, 2 * q:2 * q + 2, :],
                            start=(q == 0), stop=(q == KD // 2 - 1),
                            perf_mode=DROW,
                        )
                    if j == 2:
                        # broadcast alpha/SK across partitions (K=1 matmul)
                        pb = psb.tile([128, cc], F32, tag="ab")
                        if nchunks == 1:
                            nc.tensor.matmul(pb[:], ones_lhs[:], alr[:])
                        else:
                            nc.tensor.matmul(pb[:], ones_lhs[:], alr[:, csl])
                        nc.vector.tensor_copy(al_b[:], pb[:])
                    if j < 16:
                        nc.vector.tensor_scalar(
                            z_sb[:, j, :], pz[:], cst[:, j:j + 1], 0.0,
                            ALU.add, ALU.max,
                        )
                    else:
                        nc.scalar.activation(
                            z_sb[:, j, :], pz[:], AF.Relu,
                            bias=cst[:, j:j + 1],
                        )

                # ---- matmul 2: bf16, out = (py + b2K) * (alpha/SK) + x ----
                ch = cc // 2
                for i in range(KD):
                    xts = xtp.tile([128, cc], F32, tag="xt")
                    if nchunks == 1:
                        nc.gpsimd.dma_start(out=xts[:], in_=xt_d[:, i, :])
                    else:
                        nc.gpsimd.dma_start(out=xts[:], in_=xt_d[:, i, csl])
                    if i < 2:
                        quarters = [w2pre[(i, h)] for h in range(4)]
                    else:
                        quarters = []
                        for h in range(4):
                            w2sb = w2p.tile([128, KQ, 128], BF16, tag="w2")
                            nc.sync.dma_start(
                                out=w2sb[:],
                                in_=w2_d[i][:, h * KQ:(h + 1) * KQ, :],
                            )
                            quarters.append(w2sb)
                    py = psy.tile([128, cc], F32, tag="y")
                    for k2 in range(KF):
                        nc.tensor.matmul(
                            py[:],
                            quarters[k2 // KQ][:, k2 % KQ, :],
                            z_sb[:, k2, :],
                            start=(k2 == 0), stop=(k2 == KF - 1),
                        )
                    t2 = tmpp.tile([128, cc], F32, tag="t2")
                    nc.vector.scalar_tensor_tensor(
                        t2[:], py[:], cst[:, KF + i:KF + i + 1], al_b[:],
                        ALU.add, ALU.mult,
                    )
                    o = outp.tile([128, cc], F32, tag="o")
                    nc.gpsimd.tensor_tensor(o[:], t2[:], xts[:], ALU.add)
                    if nchunks == 1:
                        nc.gpsimd.dma_start(out=out_d[i][:, 0:ch], in_=o[:, 0:ch])
                        nc.scalar.dma_start(out=out_d[i][:, ch:cc], in_=o[:, ch:cc])
                    else:
                        lo = bass.ds(csl.start, ch)
                        hi = bass.ds(csl.start + ch, cc - ch)
                        nc.gpsimd.dma_start(out=out_d[i][:, lo], in_=o[:, 0:ch])
                        nc.scalar.dma_start(out=out_d[i][:, hi], in_=o[:, ch:cc])

    nc.compile()
    return nc


def kernel(x, centroids, w1, b1, w2, b2, gamma, beta):
    x = np.ascontiguousarray(np.asarray(x, dtype=np.float32))
    centroids = np.asarray(centroids, dtype=np.float32)
    w1 = np.asarray(w1, dtype=np.float32)
    b1 = np.asarray(b1, dtype=np.float32)
    w2 = np.asarray(w2, dtype=np.float32)
    b2 = np.asarray(b2, dtype=np.float32)
    gamma = np.asarray(gamma, dtype=np.float32)
    beta = np.asarray(beta, dtype=np.float32)

    orig_shape = x.shape
    feats = x.reshape(-1, D)
    T = feats.shape[0]

    # routing + layernorm stats + gate — same math as the reference
    aff = feats @ centroids.T
    eid = np.argmax(aff, axis=1)
    mu = feats.mean(axis=-1, keepdims=True)
    var = feats.var(axis=-1, keepdims=True)
    xhat = (feats - mu) / np.sqrt(var + EPS)
    idxs = [np.nonzero(eid == e)[0] for e in range(E)]
    counts = [len(ix) for ix in idxs]
    c_total = max(64, ((max(counts) + 7) // 8) * 8)

    nc = _build(c_total)

    in_maps = []
    for e in range(E):
        n_e = counts[e]
        xt = np.zeros((D, c_total), dtype=np.float32)
        xh = np.zeros((D, c_total), dtype=np.float32)
        alr = np.zeros((1, c_total), dtype=np.float32)
        if n_e:
            xt[:, :n_e] = feats[idxs[e]].T
            xh[:, :n_e] = xhat[idxs[e]].T
            alr[0, :n_e] = 1.0 / (1.0 + np.exp(-feats[idxs[e]] @ centroids[e])) / SK
        xt = np.ascontiguousarray(xt.reshape(KD, 128, c_total).transpose(1, 0, 2))
        xh = np.ascontiguousarray(xh.reshape(KD, 128, c_total).transpose(1, 0, 2))
        xq8 = (xh * SX).astype(ml_dtypes.float8_e4m3)

        w1e = gamma[e][:, None] * w1[e]                       # [D, F]
        b1e = b1[e] + beta[e] @ w1[e]                         # [F]
        w1q = np.ascontiguousarray(
            (w1e * SW).reshape(KD, 128, KF, 128).transpose(2, 1, 0, 3)
        ).astype(ml_dtypes.float8_e4m3)                       # [KF,128,KD,128]
        w1a = np.ascontiguousarray(w1q[:2])                   # [2,128,KD,128]
        # pack pairs of f-tiles 2..31: [NP, 128, 2*KD, 128]
        w1b = np.ascontiguousarray(
            w1q[2:].reshape(NP, 2, 128, KD, 128).transpose(0, 2, 1, 3, 4)
        ).reshape(NP, 128, 2 * KD, 128)
        w2tb = np.ascontiguousarray(
            w2[e].reshape(KF, 128, KD, 128).transpose(2, 1, 0, 3)
        ).astype(ml_dtypes.bfloat16)                          # [KD,128,KF,128]

        cst = np.empty((128, KF + KD), dtype=np.float32)
        cst[:, :KF] = (b1e * SK).reshape(KF, 128).T
        cst[:, KF:] = (b2[e] * SK).reshape(KD, 128).T
        in_maps.append(
            dict(xq=xq8, w1a=w1a, w1b=w1b, w2t=w2tb, cst=cst, alr=alr, xt=xt)
        )

    res = bass_utils.run_bass_kernel_spmd(nc, in_maps, core_ids=list(range(E)))
    kernel._last_res = res

    out = np.empty((T, D), dtype=np.float32)
    for e in range(E):
        if counts[e]:
            ye = np.asarray(res.results[e]["out"]).reshape(D, c_total)
            out[idxs[e]] = ye[:, : counts[e]].T
    return out.reshape(orig_shape)


# revision 16
# speedup vs baseline: 1.0094x; 1.0094x over previous
"""Expert-parallel BaseLayer MoE kernel for 8 TRN2 NeuronCores.

Strategy: routing (argmax over token-centroid affinities), layernorm
statistics and the sigmoid gate are computed on the host as part of the
sharding step — each core owns one expert and receives exactly the tokens
routed to it (padded to a common capacity C), pre-normalized (xhat) and
pre-transposed to [d, C].  The device does only the heavy compute:

  matmul-1 runs in fp8-e4m3 DoubleRow mode (two 128-deep k-tiles per
  instruction): pz = (64*w1')^T @ (16*xhat), epilogue
  z = max(pz + 1024*b1', 0) stored bf16 (relu commutes with the positive
  scale, which is folded into the host-side alpha/b2 constants);
  matmul-2 runs in bf16: py = w2^T z, epilogue
  out = (py + 1024*b2) * (alpha/1024) + x.

All quantization scales are powers of two so the host-side numpy
simulation is bit-identical to the device math.  Weight streams are split
across the sync and gpsimd DMA queues (w1) with the first two f-tiles as
singles for a fast start; w2 rides the sync queue as quarter-tiles once
the w1 stream drains.  No collectives; the host scatters per-expert
outputs back.
"""

import functools
import sys

import numpy as np

for _p in ("/opt/trn_rl_repo", "/opt/pypackages"):
    if _p not in sys.path:
        sys.path.append(_p)

import ml_dtypes  # noqa: E402

import concourse.bass as bass  # noqa: E402
import concourse.mybir as mybir  # noqa: E402
import concourse.tile as tile  # noqa: E402
from concourse import bacc  # noqa: E402
from concourse import bass_utils  # noqa: E402


def _ensure_axon_hooks():
    """bass_utils' trace path imports antenv.axon_hooks, which some agent
    images lack; synthesize it (with the real ctypes NTFF hook when
    available) so tracing degrades gracefully instead of crashing."""
    try:
        import antenv.axon_hooks  # noqa: F401
        return
    except ImportError:
        pass
    import types

    import antenv

    hooks = types.ModuleType("antenv.axon_hooks")
    hooks._hook = None
    hooks.set_axon_ntff_profile_hook = lambda h: setattr(hooks, "_hook", h)
    hooks.get_axon_ntff_profile_hook = lambda: hooks._hook
    sys.modules["antenv.axon_hooks"] = hooks
    antenv.axon_hooks = hooks
    try:
        from trn_agent_boot.trn_boot import _ntff_profile_via_ctypes

        hooks._hook = _ntff_profile_via_ctypes("/opt/axon/libaxon_pjrt.so")
    except Exception:
        pass


_ensure_axon_hooks()

E = 8
D = 1024
F = 4096
EPS = 1e-5
KD = D // 128   # 8 k-tiles over d
KF = F // 128   # 32 k-tiles over f
NP = KF // 2 - 1  # w1 pair-groups (f-tiles 2..31)
MAX_TC = 512    # PSUM free-dim limit for f32
SX = 16.0       # xhat fp8 scale (power of 2: lossless)
SW = 64.0       # w1 fp8 scale  (power of 2: lossless)
SK = SX * SW    # combined m1 output scale

F32 = mybir.dt.float32
BF16 = mybir.dt.bfloat16
F8 = mybir.dt.float8e4
ALU = mybir.AluOpType
AF = mybir.ActivationFunctionType
DROW = mybir.MatmulPerfMode.DoubleRow


def _chunk_slices(chunks):
    out, c0 = [], 0
    for cc in chunks:
        out.append(bass.ds(c0, cc))
        c0 += cc
    return out


def _token_chunks(c_total):
    n = (c_total + MAX_TC - 1) // MAX_TC
    base = c_total // n
    rem = c_total - base * n
    return [base + (1 if i < rem else 0) for i in range(n)]


@functools.lru_cache(maxsize=4)
def _build(c_total):
    nc = bacc.Bacc("TRN2", target_bir_lowering=False, debug=False, num_devices=E)

    # fp8 xhat (pre-scaled by SX), [128, KD, C]
    xq_d = nc.declare_dram_parameter("xq", [128, KD, c_total], F8, isOutput=False)
    # fp8 folded w1 (pre-scaled by SW): f-tiles 0,1 as singles, 2..31 as pairs
    w1a_d = nc.declare_dram_parameter("w1a", [2, 128, KD, 128], F8, isOutput=False)
    w1b_d = nc.declare_dram_parameter("w1b", [NP, 128, 2 * KD, 128], F8,
                                      isOutput=False)
    # bf16 w2, per d-tile: [KD, 128, KF, 128]
    w2_d = nc.declare_dram_parameter("w2t", [KD, 128, KF, 128], BF16, isOutput=False)
    # packed consts: col 0..KF-1 = b1K = SK*(b1 + beta@w1); col KF..KF+KD-1 = SK*b2
    cst_d = nc.declare_dram_parameter("cst", [128, KF + KD], F32, isOutput=False)
    # per-token alpha/SK row
    alr_d = nc.declare_dram_parameter("alr", [1, c_total], F32, isOutput=False)
    # f32 raw x for the residual, [128, KD, C]
    xt_d = nc.declare_dram_parameter("xt", [128, KD, c_total], F32, isOutput=False)
    out_d = nc.declare_dram_parameter("out", [KD, 128, c_total], F32, isOutput=True)

    KH = KF // 2  # w2 half width (16 k2-tiles)

    with tile.TileContext(nc) as tc:
        with (
            tc.tile_pool(name="const", bufs=1) as constp,
            tc.tile_pool(name="xqp", bufs=1) as xqp,
            tc.tile_pool(name="zp", bufs=1) as zp,
            tc.tile_pool(name="w1sp", bufs=2) as w1sp,
            tc.tile_pool(name="w1p", bufs=15) as w1p,
            tc.tile_pool(name="w2p", bufs=8) as w2p,
            tc.tile_pool(name="xtp", bufs=2) as xtp,
            tc.tile_pool(name="bcast", bufs=1) as bcastp,
            tc.tile_pool(name="tmp", bufs=2) as tmpp,
            tc.tile_pool(name="outp", bufs=3) as outp,
            tc.tile_pool(name="ps_z", bufs=5, space=bass.MemorySpace.PSUM) as psz,
            tc.tile_pool(name="ps_y", bufs=2, space=bass.MemorySpace.PSUM) as psy,
            tc.tile_pool(name="ps_b", bufs=1, space=bass.MemorySpace.PSUM) as psb,
        ):
            chunks = _token_chunks(c_total)
            slices = _chunk_slices(chunks)
            nchunks = len(chunks)

            cst = constp.tile([128, KF + KD], F32, tag="cst")
            ones_lhs = constp.tile([1, 128], F32, tag="ones")
            alr = constp.tile([1, c_total], F32, tag="alr")

            for ci, cc in enumerate(chunks):
                csl = slices[ci]
                first = ci == 0

                # ---- critical-path DMAs first: xq halves on scalar+gpsimd,
                # w1 singles on sync+gpsimd, w1 pairs alternate sync/gpsimd ----
                xq = xqp.tile([128, KD, cc], F8, tag="xq")
                w1t0 = w1sp.tile([128, KD, 128], F8, tag="w1s")
                nc.sync.dma_start(out=w1t0[:], in_=w1a_d[0])
                if first:
                    nc.scalar.dma_start(out=cst[:], in_=cst_d[:])
                    nc.scalar.dma_start(out=alr[:], in_=alr_d[:])
                    nc.vector.memset(ones_lhs[:], 1.0)
                if nchunks == 1:
                    nc.scalar.dma_start(out=xq[:, 0:KD // 2, :],
                                        in_=xq_d[:, 0:KD // 2, :])
                    nc.gpsimd.dma_start(out=xq[:, KD // 2:KD, :],
                                        in_=xq_d[:, KD // 2:KD, :])
                else:
                    nc.scalar.dma_start(out=xq[:, 0:KD // 2, :],
                                        in_=xq_d[:, 0:KD // 2, csl])
                    nc.gpsimd.dma_start(out=xq[:, KD // 2:KD, :],
                                        in_=xq_d[:, KD // 2:KD, csl])
                w1t1 = w1sp.tile([128, KD, 128], F8, tag="w1s")
                nc.gpsimd.dma_start(out=w1t1[:], in_=w1a_d[1])

                z_sb = zp.tile([128, KF, cc], BF16, tag="z")
                al_b = bcastp.tile([128, cc], F32, tag="al")

                # prefetch w2 for the first two d-tiles on the scalar queue —
                # bacc hoists m2 matmuls into the m1 stream, so their weights
                # must be resident early or the tensor queue head-blocks
                w2pre = {}
                for i in range(2):
                    for h in range(2):
                        w2sb = w2p.tile([128, KH, 128], BF16, tag="w2")
                        nc.scalar.dma_start(
                            out=w2sb[:],
                            in_=w2_d[i][:, h * KH:(h + 1) * KH, :],
                        )
                        w2pre[(i, h)] = w2sb

                # ---- matmul 1: fp8 DoubleRow, z = max(pz + b1K, 0) ----
                w1sb = None
                for j in range(KF):
                    if j < 2:
                        wt, jo = (w1t0, 0) if j == 0 else (w1t1, 0)
                    else:
                        p = (j - 2) // 2
                        if (j - 2) % 2 == 0:
                            w1sb = w1p.tile([128, 2 * KD, 128], F8, tag="w1")
                            eng = nc.sync if p % 2 == 0 else nc.gpsimd
                            eng.dma_start(out=w1sb[:], in_=w1b_d[p])
                        wt, jo = w1sb, ((j - 2) % 2) * KD
                    pz = psz.tile([128, cc], F32, tag="z")
                    for q in range(KD // 2):
                        nc.tensor.matmul(
                            pz[:],
                            wt[:, jo + 2 * q:jo + 2 * q + 2, :],
                            xq[:, 2 * q:2 * q + 2, :],
                            start=(q == 0), stop=(q == KD // 2 - 1),
                            perf_mode=DROW,
                        )
                    if j == 2:
                        # broadcast alpha/SK across partitions (K=1 matmul)
                        pb = psb.tile([128, cc], F32, tag="ab")
                        if nchunks == 1:
                            nc.tensor.matmul(pb[:], ones_lhs[:], alr[:])
                        else:
                            nc.tensor.matmul(pb[:], ones_lhs[:], alr[:, csl])
                        nc.vector.tensor_copy(al_b[:], pb[:])
                    if j < 16:
                        nc.vector.tensor_scalar(
                            z_sb[:, j, :], pz[:], cst[:, j:j + 1], 0.0,
                            ALU.add, ALU.max,
                        )
                    else:
                        nc.scalar.activation(
                            z_sb[:, j, :], pz[:], AF.Relu,
                            bias=cst[:, j:j + 1],
                        )

                # ---- matmul 2: bf16, out = (py + b2K) * (alpha/SK) + x ----
                ch = cc // 2
                for i in range(KD):
                    xts = xtp.tile([128, cc], F32, tag="xt")
                    if nchunks == 1:
                        nc.gpsimd.dma_start(out=xts[:], in_=xt_d[:, i, :])
                    else:
                        nc.gpsimd.dma_start(out=xts[:], in_=xt_d[:, i, csl])
                    if i < 2:
                        halves = [w2pre[(i, h)] for h in range(2)]
                    else:
                        halves = []
                        for h in range(2):
                            w2sb = w2p.tile([128, KH, 128], BF16, tag="w2")
                            nc.sync.dma_start(
                                out=w2sb[:],
                                in_=w2_d[i][:, h * KH:(h + 1) * KH, :],
                            )
                            halves.append(w2sb)
                    py = psy.tile([128, cc], F32, tag="y")
                    for k2 in range(KF):
                        nc.tensor.matmul(
                            py[:],
                            halves[k2 // KH][:, k2 % KH, :],
                            z_sb[:, k2, :],
                            start=(k2 == 0), stop=(k2 == KF - 1),
                        )
                    t2 = tmpp.tile([128, cc], F32, tag="t2")
                    nc.vector.scalar_tensor_tensor(
                        t2[:], py[:], cst[:, KF + i:KF + i + 1], al_b[:],
                        ALU.add, ALU.mult,
                    )
                    o = outp.tile([128, cc], F32, tag="o")
                    nc.gpsimd.tensor_tensor(o[:], t2[:], xts[:], ALU.add)
                    if nchunks == 1:
                        nc.gpsimd.dma_start(out=out_d[i][:, 0:ch], in_=o[:, 0:ch])
                        nc.scalar.dma_start(out=out_d[i][:, ch:cc], in_=o[:, ch:cc])
                    else:
                        lo = bass.ds(csl.start, ch)
                        hi = bass.ds(csl.start + ch, cc - ch)
                        nc.gpsimd.dma_start(out=out_d[i][:, lo], in_=o[:, 0:ch])
                        nc.scalar.dma_start(out=out_d[i][:, hi], in_=o[:, ch:cc])

    nc.compile()
    return nc


def kernel(x, centroids, w1, b1, w2, b2, gamma, beta):
    x = np.ascontiguousarray(np.asarray(x, dtype=np.float32))
    centroids = np.asarray(centroids, dtype=np.float32)
    w1 = np.asarray(w1, dtype=np.float32)
    b1 = np.asarray(b1, dtype=np.float32)
    w2 = np.asarray(w2, dtype=np.float32)
    b2 = np.asarray(b2, dtype=np.float32)
    gamma = np.asarray(gamma, dtype=np.float32)
    beta = np.asarray(beta, dtype=np.float32)

    orig_shape = x.shape
    feats = x.reshape(-1, D)
    T = feats.shape[0]

    # routing + layernorm stats + gate — same math as the reference
    aff = feats @ centroids.T
    eid = np.argmax(aff, axis=1)
    mu = feats.mean(axis=-1, keepdims=True)
    var = feats.var(axis=-1, keepdims=True)
    xhat = (feats - mu) / np.sqrt(var + EPS)
    idxs = [np.nonzero(eid == e)[0] for e in range(E)]
    counts = [len(ix) for ix in idxs]
    c_total = max(64, ((max(counts) + 7) // 8) * 8)

    nc = _build(c_total)

    in_maps = []
    for e in range(E):
        n_e = counts[e]
        xt = np.zeros((D, c_total), dtype=np.float32)
        xh = np.zeros((D, c_total), dtype=np.float32)
        alr = np.zeros((1, c_total), dtype=np.float32)
        if n_e:
            xt[:, :n_e] = feats[idxs[e]].T
            xh[:, :n_e] = xhat[idxs[e]].T
            alr[0, :n_e] = 1.0 / (1.0 + np.exp(-feats[idxs[e]] @ centroids[e])) / SK
        xt = np.ascontiguousarray(xt.reshape(KD, 128, c_total).transpose(1, 0, 2))
        xh = np.ascontiguousarray(xh.reshape(KD, 128, c_total).transpose(1, 0, 2))
        xq8 = (xh * SX).astype(ml_dtypes.float8_e4m3)

        w1e = gamma[e][:, None] * w1[e]                       # [D, F]
        b1e = b1[e] + beta[e] @ w1[e]                         # [F]
        w1q = np.ascontiguousarray(
            (w1e * SW).reshape(KD, 128, KF, 128).transpose(2, 1, 0, 3)
        ).astype(ml_dtypes.float8_e4m3)                       # [KF,128,KD,128]
        w1a = np.ascontiguousarray(w1q[:2])                   # [2,128,KD,128]
        # pack pairs of f-tiles 2..31: [NP, 128, 2*KD, 128]
        w1b = np.ascontiguousarray(
            w1q[2:].reshape(NP, 2, 128, KD, 128).transpose(0, 2, 1, 3, 4)
        ).reshape(NP, 128, 2 * KD, 128)
        w2tb = np.ascontiguousarray(
            w2[e].reshape(KF, 128, KD, 128).transpose(2, 1, 0, 3)
        ).astype(ml_dtypes.bfloat16)                          # [KD,128,KF,128]

        cst = np.empty((128, KF + KD), dtype=np.float32)
        cst[:, :KF] = (b1e * SK).reshape(KF, 128).T
        cst[:, KF:] = (b2[e] * SK).reshape(KD, 128).T
        in_maps.append(
            dict(xq=xq8, w1a=w1a, w1b=w1b, w2t=w2tb, cst=cst, alr=alr, xt=xt)
        )

    res = bass_utils.run_bass_kernel_spmd(nc, in_maps, core_ids=list(range(E)))
    kernel._last_res = res

    out = np.empty((T, D), dtype=np.float32)
    for e in range(E):
        if counts[e]:
            ye = np.asarray(res.results[e]["out"]).reshape(D, c_total)
            out[idxs[e]] = ye[:, : counts[e]].T
    return out.reshape(orig_shape)
